# revision 3
# baseline (speedup 1.0000x reference)
# Fused dynamic-conv (CondInst-style) + dice loss kernel for 8x TRN2 NeuronCores.
#
# Reference computation (per batch image b, object o):
#   weight[b,o,:] = conv_weight[b, :, ind[b,o]]           (gather, 593 params)
#   feat = concat(seg_feat[b], x_rel(o), y_rel(o))        ([18, 128*128])
#   h1 = relu(w1 @ feat + b1); h2 = relu(w2 @ h1 + b2)    (16-ch dynamic 1x1 convs)
#   out = sigmoid(w3 . h2 + b3)                           ([128*128])
#   dice over masked objects -> scalar loss
#
# Strategy:
#  * Host gathers the 593 dynamic params per object (303KB of the 155MB
#    conv_weight) and packs active (mask=1) objects into groups of 8.
#    x_rel/y_rel are separable: x_rel = xg/128 - x_o/128, so the grid rows are
#    object-independent and the object offsets fold into an effective bias
#    b1_eff. One shared [18, HW] feature map per image serves all its objects.
#  * A group of 8 objects => block-diagonal weights, so the three GEMMs run as
#    [K<=128, M<=128, N=512] float32r matmuls at full PE column rate.
#  * Work unit = (group, half-image) = 8192 px. Units are distributed over the
#    8 cores (SPMD single program; all data dependence lives in the inputs).
#  * gemm3 ([K=128, M=32, N=512], fp16, column-tiled) writes quadrant-aligned
#    blocks of a [128, 2048] PSUM region; lhsT3 columns 8:32 are zero so
#    filler rows are exact zeros and a -50 bias drives them to sigmoid ~= 0.
#  * Dice partials per unit via fused ops: tensor_tensor_reduce (sum pred*tgt)
#    and Square activation with accum_out (sum pred^2); the host does the
#    final tiny reduction plus sum(tgt^2), which is network-independent.
import numpy as np
from contextlib import ExitStack

import concourse.bass as bass
import concourse.tile as tile
from concourse import mybir, bacc
from concourse.bass_utils import run_bass_kernel_spmd

C = 16
WT = 593
B, O, H, W = 4, 32, 128, 128
HW = H * W
N_CORES = 8
GRP = 8            # objects per block-diagonal group
HALF = 8192        # pixels per work unit
NT = HALF // 512   # 512-px moving-tiles per unit (16)

F32 = mybir.dt.float32
F32R = mybir.dt.float32r
F16 = mybir.dt.float16
ACTF = mybir.ActivationFunctionType
ALU = mybir.AluOpType

# wpack free-dim layout (per unit, [128, 259] float32r):
#   0:128   lhsT2 (block-diag w2^T)
#   128     b2
#   129     b3 (sigmoid bias; valid rows 32q+oo, filler rows -50)
#   130     b1_eff
#   131:259 lhsT1 (w1^T incl. grid channels; rows 0:18 used)
WCOLS = 259


def host_pack(seg_feat, conv_weight, mask, ind, target):
    cw = conv_weight.reshape(B, WT, HW)
    weight = np.take_along_axis(cw, ind[:, None, :].astype(np.int64), axis=2)
    weight = np.ascontiguousarray(weight.transpose(0, 2, 1))  # [B, O, WT]
    s0 = (C + 2) * C
    w1 = weight[..., :s0].reshape(B, O, C, C + 2)
    b1 = weight[..., s0:s0 + C]
    w2 = weight[..., s0 + C:s0 + C + C * C].reshape(B, O, C, C)
    b2 = weight[..., s0 + C + C * C:s0 + 2 * C + C * C]
    w3 = weight[..., s0 + 2 * C + C * C:s0 + 3 * C + C * C]
    b3 = weight[..., -1]
    xo = (ind % W).astype(np.float32)
    yo = (ind // W).astype(np.float32)

    units = []  # (b, objs[8 padded with -1], half)
    for b in range(B):
        objs = [o for o in range(O) if mask[b, o] == 1]
        for g0 in range(0, len(objs), GRP):
            grp = objs[g0:g0 + GRP]
            grp = grp + [-1] * (GRP - len(grp))
            for half in range(2):
                units.append((b, grp, half))
    per_core = [[] for _ in range(N_CORES)]
    for i, u in enumerate(units):
        per_core[i % N_CORES].append(u)
    NG = max(1, max(len(u) for u in per_core))
    for ci in range(N_CORES):
        while len(per_core[ci]) < NG:
            per_core[ci].append((0, [-1] * GRP, 0))

    px = np.arange(HW, dtype=np.float32)
    xg = (px % W) / 128.0
    yg = np.floor(px / W) / 128.0
    tgt_flat = target.reshape(B, O, HW)

    in_maps = []
    for ci in range(N_CORES):
        feat_pack = np.zeros((NG, 18, HALF), np.float32)
        wpack = np.zeros((NG, 128, WCOLS), np.float32)
        w3pack = np.zeros((NG, 128, 32), np.float16)
        tgt_pack = np.zeros((NG, 128, 2048), np.float32)
        for u, (b, grp, half) in enumerate(per_core[ci]):
            sl = slice(half * HALF, (half + 1) * HALF)
            feat_pack[u, :16] = seg_feat[b].reshape(C, HW)[:, sl]
            feat_pack[u, 16] = xg[sl]
            feat_pack[u, 17] = yg[sl]
            wpack[u, :, 129] = -50.0  # filler-row sigmoid bias
            for oo, o in enumerate(grp):
                if o < 0:
                    continue
                wpack[u, :18, 131 + 16 * oo:131 + 16 * oo + 16] = w1[b, o].T
                b1e = (b1[b, o] - w1[b, o, :, 16] * (xo[b, o] / 128.0)
                       - w1[b, o, :, 17] * (yo[b, o] / 128.0))
                wpack[u, 16 * oo:16 * oo + 16, 130] = b1e
                wpack[u, 16 * oo:16 * oo + 16, 16 * oo:16 * oo + 16] = w2[b, o].T
                wpack[u, 16 * oo:16 * oo + 16, 128] = b2[b, o]
                w3pack[u, 16 * oo:16 * oo + 16, oo] = w3[b, o].astype(np.float16)
                for q in range(4):
                    wpack[u, 32 * q + oo, 129] = b3[b, o]
                # tgt in the packed sigmoid layout: partition 32q+oo holds
                # moving-tiles t = 4k+q at free cols 512k..512k+512
                for t in range(NT):
                    q, k = t % 4, t // 4
                    g0 = half * HALF + t * 512
                    tgt_pack[u, 32 * q + oo, 512 * k:512 * k + 512] = \
                        tgt_flat[b, o, g0:g0 + 512]
        in_maps.append({"feat": feat_pack, "wpack": wpack,
                        "w3pack": w3pack, "tgt": tgt_pack})
    return in_maps, per_core, NG


_PROGRAM_CACHE = {}


def build_program(NG):
    if NG in _PROGRAM_CACHE:
        return _PROGRAM_CACHE[NG]
    nc = bacc.Bacc("TRN2", target_bir_lowering=False, debug=False,
                   enable_asserts=False, num_devices=N_CORES)
    feat_t = nc.dram_tensor("feat", (NG, 18, HALF), F32R, kind="ExternalInput")
    wpack_t = nc.dram_tensor("wpack", (NG, 128, WCOLS), F32R, kind="ExternalInput")
    w3pack_t = nc.dram_tensor("w3pack", (NG, 128, 32), F16, kind="ExternalInput")
    tgt_t = nc.dram_tensor("tgt", (NG, 128, 2048), F32, kind="ExternalInput")
    acc_t = nc.dram_tensor("acc", (2, 128, NG), F32, kind="ExternalOutput")

    with tile.TileContext(nc) as tc, ExitStack() as ctx:
        wpool = ctx.enter_context(tc.tile_pool(name="wpool", bufs=2))
        fpool = ctx.enter_context(tc.tile_pool(name="fpool", bufs=2))
        h1pool = ctx.enter_context(tc.tile_pool(name="h1pool", bufs=1))
        h2pool = ctx.enter_context(tc.tile_pool(name="h2pool", bufs=1))
        tpool = ctx.enter_context(tc.tile_pool(name="tpool", bufs=2))
        ppool = ctx.enter_context(tc.tile_pool(name="ppool", bufs=2))
        spool = ctx.enter_context(tc.tile_pool(name="spool", bufs=2))
        apool = ctx.enter_context(tc.tile_pool(name="apool", bufs=1))
        ps = ctx.enter_context(tc.tile_pool(name="ps", bufs=2, space="PSUM"))

        inter_acc = apool.tile([128, NG], F32)
        psq_acc = apool.tile([128, NG], F32)

        for u in range(NG):
            wt = wpool.tile([128, WCOLS], F32R, tag="w")
            nc.sync.dma_start(out=wt, in_=wpack_t.ap()[u])
            w3t = wpool.tile([128, 32], F16, tag="w3")
            nc.sync.dma_start(out=w3t, in_=w3pack_t.ap()[u])
            ft = fpool.tile([18, HALF], F32R, tag="f")
            nc.sync.dma_start(out=ft, in_=feat_t.ap()[u])
            tg = tpool.tile([128, 2048], F32, tag="t")
            nc.sync.dma_start(out=tg, in_=tgt_t.ap()[u])

            lhsT2 = wt[:, 0:128]
            b2ap = wt[:, 128:129].bitcast(F32)
            b3ap = wt[:, 129:130].bitcast(F32)
            b1ap = wt[:, 130:131].bitcast(F32)
            lhsT1 = wt[0:18, 131:WCOLS]

            h1 = h1pool.tile([128, HALF], F32R, tag="h1")
            h2 = h2pool.tile([128, HALF], F16, tag="h2")

            # phase A: gemm1 + relu1(+bias); evacuations alternate ACT/DVE
            for j in range(4):
                pa = ps.tile([128, 2048], F32, tag="ps")
                for s in range(4):
                    t = 4 * j + s
                    nc.tensor.matmul(
                        pa[:, 512 * s:512 * s + 512], lhsT1,
                        ft[:, 512 * t:512 * t + 512],
                        start=True, stop=True)
                dst = h1[:, 2048 * j:2048 * j + 2048]
                if j % 2 == 0:
                    nc.scalar.activation(dst, pa, ACTF.Relu, bias=b1ap, scale=1.0)
                else:
                    nc.vector.tensor_scalar(out=dst, in0=pa, scalar1=b1ap,
                                            scalar2=0.0, op0=ALU.add, op1=ALU.max)
            # phase B: gemm2 + relu2(+bias), h2 in fp16
            for j in range(4):
                pb = ps.tile([128, 2048], F32, tag="ps")
                for s in range(4):
                    t = 4 * j + s
                    nc.tensor.matmul(
                        pb[:, 512 * s:512 * s + 512], lhsT2,
                        h1[:, 512 * t:512 * t + 512],
                        start=True, stop=True)
                dst = h2[:, 2048 * j:2048 * j + 2048]
                if j % 2 == 1:
                    nc.scalar.activation(dst, pb, ACTF.Relu, bias=b2ap, scale=1.0)
                else:
                    nc.vector.tensor_scalar(out=dst, in0=pb, scalar1=b2ap,
                                            scalar2=0.0, op0=ALU.add, op1=ALU.max)
            # phase C: gemm3 column-tiled (fp16), quadrant-packed [128, 2048]
            pc = ps.tile([128, 2048], F32, tag="ps")
            for t in range(NT):
                q, k = t % 4, t // 4
                nc.tensor.matmul(pc[32 * q:32 * q + 32, 512 * k:512 * k + 512],
                                 w3t, h2[:, 512 * t:512 * t + 512],
                                 start=True, stop=True, tile_position=(0, 32 * q))
            pred = ppool.tile([128, 2048], F32, tag="p")
            nc.scalar.activation(pred, pc, ACTF.Sigmoid, bias=b3ap, scale=1.0)
            sq = spool.tile([128, 2048], F32, tag="s")
            nc.scalar.activation(sq, pred, ACTF.Square,
                                 accum_out=psq_acc[:, u:u + 1])
            prod = spool.tile([128, 2048], F32, tag="s")
            nc.vector.scalar_tensor_tensor(
                out=prod, in0=pred, scalar=0.0, in1=tg,
                op0=ALU.add, op1=ALU.mult, accum_out=inter_acc[:, u:u + 1])

        nc.sync.dma_start(out=acc_t.ap()[0], in_=inter_acc)
        nc.sync.dma_start(out=acc_t.ap()[1], in_=psq_acc)

    nc.compile()
    _PROGRAM_CACHE[NG] = nc
    return nc


def _run(inputs, trace=False):
    seg_feat = np.asarray(inputs["seg_feat"], np.float32)
    conv_weight = np.asarray(inputs["conv_weight"], np.float32)
    mask = np.asarray(inputs["mask"])
    ind = np.asarray(inputs["ind"])
    target = np.asarray(inputs["target"], np.float32)

    in_maps, per_core, NG = host_pack(seg_feat, conv_weight, mask, ind, target)
    nc = build_program(NG)
    res = run_bass_kernel_spmd(nc, in_maps, core_ids=list(range(N_CORES)),
                               trace=trace)

    inter = np.zeros(B, np.float64)
    predsq = np.zeros(B, np.float64)
    for ci in range(N_CORES):
        acc = res.results[ci]["acc"]
        for u, (b, grp, half) in enumerate(per_core[ci]):
            if all(o < 0 for o in grp):
                continue
            inter[b] += acc[0, :, u].sum(dtype=np.float64)
            predsq[b] += acc[1, :, u].sum(dtype=np.float64)
    tgtsq = ((target.reshape(B, O, HW).astype(np.float64) ** 2)
             * mask[:, :, None]).sum(axis=(1, 2))
    loss = 1.0 - (2.0 * inter + 1.0) / (predsq + tgtsq + 1.0)
    return np.float32(loss.mean()), res


def kernel(**inputs):
    loss, _ = _run(inputs, trace=False)
    return np.array(loss, dtype=np.float32)


# revision 4
# speedup vs baseline: 1.0783x; 1.0783x over previous
# Fused dynamic-conv (CondInst-style) + dice loss kernel for 8x TRN2 NeuronCores.
#
# Reference computation (per batch image b, object o):
#   weight[b,o,:] = conv_weight[b, :, ind[b,o]]           (gather, 593 params)
#   feat = concat(seg_feat[b], x_rel(o), y_rel(o))        ([18, 128*128])
#   h1 = relu(w1 @ feat + b1); h2 = relu(w2 @ h1 + b2)    (16-ch dynamic 1x1 convs)
#   out = sigmoid(w3 . h2 + b3)                           ([128*128])
#   dice over masked objects -> scalar loss
#
# Strategy:
#  * Host gathers the 593 dynamic params per object (303KB of the 155MB
#    conv_weight) and packs active (mask=1) objects into groups of 8.
#    x_rel/y_rel are separable: x_rel = xg/128 - x_o/128, so the grid rows are
#    object-independent and the object offsets fold into an effective bias
#    b1_eff. One shared [18, HW] feature map per image serves all its objects.
#  * A group of 8 objects => block-diagonal weights, so the three GEMMs run as
#    [K<=128, M<=128, N=512] fp16 matmuls at full PE column rate.
#  * Work unit = (group, half-image) = 8192 px. Units are distributed over the
#    8 cores (SPMD single program; all data dependence lives in the inputs).
#  * gemm3 ([K=128, M=32, N=512], fp16, column-tiled) writes quadrant-aligned
#    blocks of a [128, 2048] PSUM region; lhsT3 columns 8:32 are zero so
#    filler rows are exact zeros and a -50 bias drives them to sigmoid ~= 0.
#  * Dice partials per unit via fused ops: scalar_tensor_tensor (sum pred*tgt)
#    and Square activation with accum_out (sum pred^2); the host does the
#    final tiny reduction plus sum(tgt^2), which is network-independent.
import numpy as np
from contextlib import ExitStack

import concourse.bass as bass
import concourse.tile as tile
from concourse import mybir, bacc
from concourse.bass_utils import run_bass_kernel_spmd

C = 16
WT = 593
B, O, H, W = 4, 32, 128, 128
HW = H * W
N_CORES = 8
GRP = 8            # objects per block-diagonal group
HALF = 8192        # pixels per work unit
NT = HALF // 512   # 512-px moving-tiles per unit (16)

F32 = mybir.dt.float32
F32R = mybir.dt.float32r
F16 = mybir.dt.float16
ACTF = mybir.ActivationFunctionType
ALU = mybir.AluOpType

# wpack free-dim layout (per unit, [128, 288] float16):
#   0:128   lhsT2 (block-diag w2^T)
#   128:160 lhsT3 (block-diag w3; cols 8:32 zero)
#   160:288 lhsT1 (w1^T incl. grid channels; rows 0:18 used)
# bias layout (per unit, [128, 3] float32): 0 = b1_eff, 1 = b2, 2 = b3/-50
WCOLS = 288


def host_pack(seg_feat, conv_weight, mask, ind, target):
    cw = conv_weight.reshape(B, WT, HW)
    weight = np.take_along_axis(cw, ind[:, None, :].astype(np.int64), axis=2)
    weight = np.ascontiguousarray(weight.transpose(0, 2, 1))  # [B, O, WT]
    s0 = (C + 2) * C
    w1 = weight[..., :s0].reshape(B, O, C, C + 2)
    b1 = weight[..., s0:s0 + C]
    w2 = weight[..., s0 + C:s0 + C + C * C].reshape(B, O, C, C)
    b2 = weight[..., s0 + C + C * C:s0 + 2 * C + C * C]
    w3 = weight[..., s0 + 2 * C + C * C:s0 + 3 * C + C * C]
    b3 = weight[..., -1]
    xo = (ind % W).astype(np.float32)
    yo = (ind // W).astype(np.float32)

    units = []  # (b, objs[8 padded with -1], half)
    for b in range(B):
        objs = [o for o in range(O) if mask[b, o] == 1]
        for g0 in range(0, len(objs), GRP):
            grp = objs[g0:g0 + GRP]
            grp = grp + [-1] * (GRP - len(grp))
            for half in range(2):
                units.append((b, grp, half))
    per_core = [[] for _ in range(N_CORES)]
    for i, u in enumerate(units):
        per_core[i % N_CORES].append(u)
    NG = max(1, max(len(u) for u in per_core))
    for ci in range(N_CORES):
        while len(per_core[ci]) < NG:
            per_core[ci].append((0, [-1] * GRP, 0))

    px = np.arange(HW, dtype=np.float32)
    xg = (px % W) / 128.0
    yg = np.floor(px / W) / 128.0
    tgt_flat = target.reshape(B, O, HW)

    in_maps = []
    for ci in range(N_CORES):
        feat_pack = np.zeros((NG, 18, HALF), np.float16)
        wpack = np.zeros((NG, 128, WCOLS), np.float16)
        bias_pack = np.zeros((NG, 128, 3), np.float32)
        tgt_pack = np.zeros((NG, 128, 2048), np.float16)
        for u, (b, grp, half) in enumerate(per_core[ci]):
            sl = slice(half * HALF, (half + 1) * HALF)
            feat_pack[u, :16] = seg_feat[b].reshape(C, HW)[:, sl]
            feat_pack[u, 16] = xg[sl]
            feat_pack[u, 17] = yg[sl]
            bias_pack[u, :, 2] = -50.0  # filler-row sigmoid bias
            for oo, o in enumerate(grp):
                if o < 0:
                    continue
                wpack[u, :18, 160 + 16 * oo:160 + 16 * oo + 16] = \
                    w1[b, o].T.astype(np.float16)
                b1e = (b1[b, o] - w1[b, o, :, 16] * (xo[b, o] / 128.0)
                       - w1[b, o, :, 17] * (yo[b, o] / 128.0))
                bias_pack[u, 16 * oo:16 * oo + 16, 0] = b1e
                wpack[u, 16 * oo:16 * oo + 16, 16 * oo:16 * oo + 16] = \
                    w2[b, o].T.astype(np.float16)
                bias_pack[u, 16 * oo:16 * oo + 16, 1] = b2[b, o]
                wpack[u, 16 * oo:16 * oo + 16, 128 + oo] = \
                    w3[b, o].astype(np.float16)
                for q in range(4):
                    bias_pack[u, 32 * q + oo, 2] = b3[b, o]
                # tgt in the packed sigmoid layout: partition 32q+oo holds
                # moving-tiles t = 4k+q at free cols 512k..512k+512
                for t in range(NT):
                    q, k = t % 4, t // 4
                    g0 = half * HALF + t * 512
                    tgt_pack[u, 32 * q + oo, 512 * k:512 * k + 512] = \
                        tgt_flat[b, o, g0:g0 + 512].astype(np.float16)
        in_maps.append({"feat": feat_pack, "wpack": wpack,
                        "bias": bias_pack, "tgt": tgt_pack})
    return in_maps, per_core, NG


_PROGRAM_CACHE = {}


def build_program(NG):
    if NG in _PROGRAM_CACHE:
        return _PROGRAM_CACHE[NG]
    nc = bacc.Bacc("TRN2", target_bir_lowering=False, debug=False,
                   enable_asserts=False, num_devices=N_CORES)
    feat_t = nc.dram_tensor("feat", (NG, 18, HALF), F16, kind="ExternalInput")
    wpack_t = nc.dram_tensor("wpack", (NG, 128, WCOLS), F16, kind="ExternalInput")
    bias_t = nc.dram_tensor("bias", (NG, 128, 3), F32, kind="ExternalInput")
    tgt_t = nc.dram_tensor("tgt", (NG, 128, 2048), F16, kind="ExternalInput")
    acc_t = nc.dram_tensor("acc", (2, 128, NG), F32, kind="ExternalOutput")

    with tile.TileContext(nc) as tc, ExitStack() as ctx:
        wpool = ctx.enter_context(tc.tile_pool(name="wpool", bufs=2))
        fpool = ctx.enter_context(tc.tile_pool(name="fpool", bufs=2))
        h1pool = ctx.enter_context(tc.tile_pool(name="h1pool", bufs=1))
        h2pool = ctx.enter_context(tc.tile_pool(name="h2pool", bufs=1))
        tpool = ctx.enter_context(tc.tile_pool(name="tpool", bufs=2))
        ppool = ctx.enter_context(tc.tile_pool(name="ppool", bufs=2))
        spool = ctx.enter_context(tc.tile_pool(name="spool", bufs=2))
        apool = ctx.enter_context(tc.tile_pool(name="apool", bufs=1))
        ps = ctx.enter_context(tc.tile_pool(name="ps", bufs=2, space="PSUM"))

        inter_acc = apool.tile([128, NG], F32)
        psq_acc = apool.tile([128, NG], F32)

        for u in range(NG):
            wt = wpool.tile([128, WCOLS], F16, tag="w")
            nc.sync.dma_start(out=wt, in_=wpack_t.ap()[u])
            bt = wpool.tile([128, 3], F32, tag="b")
            nc.sync.dma_start(out=bt, in_=bias_t.ap()[u])
            ft = fpool.tile([18, HALF], F16, tag="f")
            nc.sync.dma_start(out=ft, in_=feat_t.ap()[u])
            tg = tpool.tile([128, 2048], F16, tag="t")
            nc.sync.dma_start(out=tg, in_=tgt_t.ap()[u])

            lhsT2 = wt[:, 0:128]
            w3t = wt[:, 128:160]
            lhsT1 = wt[0:18, 160:WCOLS]
            b1ap = bt[:, 0:1]
            b2ap = bt[:, 1:2]
            b3ap = bt[:, 2:3]

            h1 = h1pool.tile([128, HALF], F16, tag="h1")
            h2 = h2pool.tile([128, HALF], F16, tag="h2")

            # phase A: gemm1 + relu1(+bias); evacuations alternate ACT/DVE
            for j in range(4):
                pa = ps.tile([128, 2048], F32, tag="ps")
                for s in range(4):
                    t = 4 * j + s
                    nc.tensor.matmul(
                        pa[:, 512 * s:512 * s + 512], lhsT1,
                        ft[:, 512 * t:512 * t + 512],
                        start=True, stop=True)
                dst = h1[:, 2048 * j:2048 * j + 2048]
                if j % 2 == 0:
                    nc.scalar.activation(dst, pa, ACTF.Relu, bias=b1ap, scale=1.0)
                else:
                    nc.vector.tensor_scalar(out=dst, in0=pa, scalar1=b1ap,
                                            scalar2=0.0, op0=ALU.add, op1=ALU.max)
            # phase B: gemm2 + relu2(+bias), h2 in fp16
            for j in range(4):
                pb = ps.tile([128, 2048], F32, tag="ps")
                for s in range(4):
                    t = 4 * j + s
                    nc.tensor.matmul(
                        pb[:, 512 * s:512 * s + 512], lhsT2,
                        h1[:, 512 * t:512 * t + 512],
                        start=True, stop=True)
                dst = h2[:, 2048 * j:2048 * j + 2048]
                if j % 2 == 1:
                    nc.scalar.activation(dst, pb, ACTF.Relu, bias=b2ap, scale=1.0)
                else:
                    nc.vector.tensor_scalar(out=dst, in0=pb, scalar1=b2ap,
                                            scalar2=0.0, op0=ALU.add, op1=ALU.max)
            # phase C: gemm3 column-tiled (fp16), quadrant-packed [128, 2048]
            pc = ps.tile([128, 2048], F32, tag="ps")
            for t in range(NT):
                q, k = t % 4, t // 4
                nc.tensor.matmul(pc[32 * q:32 * q + 32, 512 * k:512 * k + 512],
                                 w3t, h2[:, 512 * t:512 * t + 512],
                                 start=True, stop=True, tile_position=(0, 32 * q))
            pred = ppool.tile([128, 2048], F32, tag="p")
            nc.scalar.activation(pred, pc, ACTF.Sigmoid, bias=b3ap, scale=1.0)
            sq = spool.tile([128, 2048], F32, tag="s")
            nc.scalar.activation(sq, pred, ACTF.Square,
                                 accum_out=psq_acc[:, u:u + 1])
            prod = spool.tile([128, 2048], F32, tag="s")
            nc.vector.scalar_tensor_tensor(
                out=prod, in0=pred, scalar=0.0, in1=tg,
                op0=ALU.add, op1=ALU.mult, accum_out=inter_acc[:, u:u + 1])

        nc.sync.dma_start(out=acc_t.ap()[0], in_=inter_acc)
        nc.sync.dma_start(out=acc_t.ap()[1], in_=psq_acc)

    nc.compile()
    _PROGRAM_CACHE[NG] = nc
    return nc


def _run(inputs, trace=False):
    seg_feat = np.asarray(inputs["seg_feat"], np.float32)
    conv_weight = np.asarray(inputs["conv_weight"], np.float32)
    mask = np.asarray(inputs["mask"])
    ind = np.asarray(inputs["ind"])
    target = np.asarray(inputs["target"], np.float32)

    in_maps, per_core, NG = host_pack(seg_feat, conv_weight, mask, ind, target)
    nc = build_program(NG)
    res = run_bass_kernel_spmd(nc, in_maps, core_ids=list(range(N_CORES)),
                               trace=trace)

    inter = np.zeros(B, np.float64)
    predsq = np.zeros(B, np.float64)
    for ci in range(N_CORES):
        acc = res.results[ci]["acc"]
        for u, (b, grp, half) in enumerate(per_core[ci]):
            if all(o < 0 for o in grp):
                continue
            inter[b] += acc[0, :, u].sum(dtype=np.float64)
            predsq[b] += acc[1, :, u].sum(dtype=np.float64)
    tgtsq = ((target.reshape(B, O, HW).astype(np.float64) ** 2)
             * mask[:, :, None]).sum(axis=(1, 2))
    loss = 1.0 - (2.0 * inter + 1.0) / (predsq + tgtsq + 1.0)
    return np.float32(loss.mean()), res


def kernel(**inputs):
    loss, _ = _run(inputs, trace=False)
    return np.array(loss, dtype=np.float32)


# revision 5
# speedup vs baseline: 1.0912x; 1.0119x over previous
# Fused dynamic-conv (CondInst-style) + dice loss kernel for 8x TRN2 NeuronCores.
#
# Reference computation (per batch image b, object o):
#   weight[b,o,:] = conv_weight[b, :, ind[b,o]]           (gather, 593 params)
#   feat = concat(seg_feat[b], x_rel(o), y_rel(o))        ([18, 128*128])
#   h1 = relu(w1 @ feat + b1); h2 = relu(w2 @ h1 + b2)    (16-ch dynamic 1x1 convs)
#   out = sigmoid(w3 . h2 + b3)                           ([128*128])
#   dice over masked objects -> scalar loss
#
# Strategy:
#  * Host gathers the 593 dynamic params per object (303KB of the 155MB
#    conv_weight) and packs active (mask=1) objects into groups of 8.
#    x_rel/y_rel are separable: x_rel = xg/128 - x_o/128, so the grid rows are
#    object-independent and the object offsets fold into an effective bias
#    b1_eff. One shared [18, HW] feature map per image serves all its objects.
#  * A group of 8 objects => block-diagonal weights, so the three GEMMs run as
#    [K<=128, M<=128, N=512] fp16 matmuls at full PE column rate.
#  * Work unit = (group, half-image) = 8192 px. Units are distributed over the
#    8 cores (SPMD single program; all data dependence lives in the inputs).
#  * gemm3 ([K=128, M=32, N=512], fp16, column-tiled) writes quadrant-aligned
#    blocks of a [128, 2048] PSUM region; lhsT3 columns 8:32 are zero so
#    filler rows are exact zeros and a -50 bias drives them to sigmoid ~= 0.
#  * Dice partials per unit via fused ops: scalar_tensor_tensor (sum pred*tgt)
#    and Square activation with accum_out (sum pred^2); the host does the
#    final tiny reduction plus sum(tgt^2), which is network-independent.
import numpy as np
from contextlib import ExitStack

import concourse.bass as bass
import concourse.tile as tile
from concourse import mybir, bacc
from concourse.bass_utils import run_bass_kernel_spmd

C = 16
WT = 593
B, O, H, W = 4, 32, 128, 128
HW = H * W
N_CORES = 8
GRP = 8            # objects per block-diagonal group
HALF = 8192        # pixels per work unit
NT = HALF // 512   # 512-px moving-tiles per unit (16)

F32 = mybir.dt.float32
F32R = mybir.dt.float32r
F16 = mybir.dt.float16
ACTF = mybir.ActivationFunctionType
ALU = mybir.AluOpType

# wpack free-dim layout (per unit, [128, 288] float16):
#   0:128   lhsT2 (block-diag w2^T)
#   128:160 lhsT3 (block-diag w3; cols 8:32 zero)
#   160:288 lhsT1 (w1^T incl. grid channels; rows 0:18 used)
# bias layout (per unit, [128, 3] float32): 0 = b1_eff, 1 = b2, 2 = b3/-50
WCOLS = 288


def host_pack(seg_feat, conv_weight, mask, ind, target):
    cw = conv_weight.reshape(B, WT, HW)
    weight = np.take_along_axis(cw, ind[:, None, :].astype(np.int64), axis=2)
    weight = np.ascontiguousarray(weight.transpose(0, 2, 1))  # [B, O, WT]
    s0 = (C + 2) * C
    w1 = weight[..., :s0].reshape(B, O, C, C + 2)
    b1 = weight[..., s0:s0 + C]
    w2 = weight[..., s0 + C:s0 + C + C * C].reshape(B, O, C, C)
    b2 = weight[..., s0 + C + C * C:s0 + 2 * C + C * C]
    w3 = weight[..., s0 + 2 * C + C * C:s0 + 3 * C + C * C]
    b3 = weight[..., -1]
    xo = (ind % W).astype(np.float32)
    yo = (ind // W).astype(np.float32)

    units = []  # (b, objs[8 padded with -1], half)
    for b in range(B):
        objs = [o for o in range(O) if mask[b, o] == 1]
        for g0 in range(0, len(objs), GRP):
            grp = objs[g0:g0 + GRP]
            grp = grp + [-1] * (GRP - len(grp))
            for half in range(2):
                units.append((b, grp, half))
    per_core = [[] for _ in range(N_CORES)]
    for i, u in enumerate(units):
        per_core[i % N_CORES].append(u)
    NG = max(1, max(len(u) for u in per_core))
    for ci in range(N_CORES):
        while len(per_core[ci]) < NG:
            per_core[ci].append((0, [-1] * GRP, 0))

    px = np.arange(HW, dtype=np.float32)
    xg = (px % W) / 128.0
    yg = np.floor(px / W) / 128.0
    tgt_flat = target.reshape(B, O, HW)

    in_maps = []
    for ci in range(N_CORES):
        feat_pack = np.zeros((NG, 18, HALF), np.float16)
        wpack = np.zeros((NG, 128, WCOLS), np.float16)
        bias_pack = np.zeros((NG, 128, 3), np.float32)
        tgt_pack = np.zeros((NG, 128, 2048), np.float16)
        for u, (b, grp, half) in enumerate(per_core[ci]):
            sl = slice(half * HALF, (half + 1) * HALF)
            feat_pack[u, :16] = seg_feat[b].reshape(C, HW)[:, sl]
            feat_pack[u, 16] = xg[sl]
            feat_pack[u, 17] = yg[sl]
            bias_pack[u, :, 2] = -50.0  # filler-row sigmoid bias
            for oo, o in enumerate(grp):
                if o < 0:
                    continue
                wpack[u, :18, 160 + 16 * oo:160 + 16 * oo + 16] = \
                    w1[b, o].T.astype(np.float16)
                b1e = (b1[b, o] - w1[b, o, :, 16] * (xo[b, o] / 128.0)
                       - w1[b, o, :, 17] * (yo[b, o] / 128.0))
                bias_pack[u, 16 * oo:16 * oo + 16, 0] = b1e
                wpack[u, 16 * oo:16 * oo + 16, 16 * oo:16 * oo + 16] = \
                    w2[b, o].T.astype(np.float16)
                bias_pack[u, 16 * oo:16 * oo + 16, 1] = b2[b, o]
                wpack[u, 16 * oo:16 * oo + 16, 128 + oo] = \
                    w3[b, o].astype(np.float16)
                for q in range(4):
                    bias_pack[u, 32 * q + oo, 2] = b3[b, o]
                # tgt in the packed sigmoid layout: partition 32q+oo holds
                # moving-tiles t = 4k+q at free cols 512k..512k+512
                for t in range(NT):
                    q, k = t % 4, t // 4
                    g0 = half * HALF + t * 512
                    tgt_pack[u, 32 * q + oo, 512 * k:512 * k + 512] = \
                        tgt_flat[b, o, g0:g0 + 512].astype(np.float16)
        in_maps.append({"feat": feat_pack, "wpack": wpack,
                        "bias": bias_pack, "tgt": tgt_pack})
    return in_maps, per_core, NG


_PROGRAM_CACHE = {}


def build_program(NG):
    if NG in _PROGRAM_CACHE:
        return _PROGRAM_CACHE[NG]
    nc = bacc.Bacc("TRN2", target_bir_lowering=False, debug=False,
                   enable_asserts=False, num_devices=N_CORES)
    feat_t = nc.dram_tensor("feat", (NG, 18, HALF), F16, kind="ExternalInput")
    wpack_t = nc.dram_tensor("wpack", (NG, 128, WCOLS), F16, kind="ExternalInput")
    bias_t = nc.dram_tensor("bias", (NG, 128, 3), F32, kind="ExternalInput")
    tgt_t = nc.dram_tensor("tgt", (NG, 128, 2048), F16, kind="ExternalInput")
    acc_t = nc.dram_tensor("acc", (2, 128, NG), F32, kind="ExternalOutput")

    with tile.TileContext(nc) as tc, ExitStack() as ctx:
        wpool = ctx.enter_context(tc.tile_pool(name="wpool", bufs=2))
        fpool = ctx.enter_context(tc.tile_pool(name="fpool", bufs=2))
        h1pool = ctx.enter_context(tc.tile_pool(name="h1pool", bufs=2))
        h2pool = ctx.enter_context(tc.tile_pool(name="h2pool", bufs=2))
        tpool = ctx.enter_context(tc.tile_pool(name="tpool", bufs=2))
        ppool = ctx.enter_context(tc.tile_pool(name="ppool", bufs=2))
        spool = ctx.enter_context(tc.tile_pool(name="spool", bufs=2))
        apool = ctx.enter_context(tc.tile_pool(name="apool", bufs=1))
        ps = ctx.enter_context(tc.tile_pool(name="ps", bufs=2, space="PSUM"))

        inter_acc = apool.tile([128, NG], F32)
        psq_acc = apool.tile([128, NG], F32)

        for u in range(NG):
            wt = wpool.tile([128, WCOLS], F16, tag="w")
            nc.sync.dma_start(out=wt, in_=wpack_t.ap()[u])
            bt = wpool.tile([128, 3], F32, tag="b")
            nc.sync.dma_start(out=bt, in_=bias_t.ap()[u])
            ft = fpool.tile([18, HALF], F16, tag="f")
            nc.sync.dma_start(out=ft, in_=feat_t.ap()[u])
            tg = tpool.tile([128, 2048], F16, tag="t")
            nc.sync.dma_start(out=tg, in_=tgt_t.ap()[u])

            lhsT2 = wt[:, 0:128]
            w3t = wt[:, 128:160]
            lhsT1 = wt[0:18, 160:WCOLS]
            b1ap = bt[:, 0:1]
            b2ap = bt[:, 1:2]
            b3ap = bt[:, 2:3]

            h1 = h1pool.tile([128, HALF], F16, tag="h1")
            h2 = h2pool.tile([128, HALF], F16, tag="h2")

            # phase A: gemm1 + relu1(+bias); evacuations alternate ACT/DVE
            for j in range(4):
                pa = ps.tile([128, 2048], F32, tag="ps")
                for s in range(4):
                    t = 4 * j + s
                    nc.tensor.matmul(
                        pa[:, 512 * s:512 * s + 512], lhsT1,
                        ft[:, 512 * t:512 * t + 512],
                        start=True, stop=True)
                dst = h1[:, 2048 * j:2048 * j + 2048]
                if j % 2 == 0:
                    nc.scalar.activation(dst, pa, ACTF.Relu, bias=b1ap, scale=1.0)
                else:
                    nc.vector.tensor_scalar(out=dst, in0=pa, scalar1=b1ap,
                                            scalar2=0.0, op0=ALU.add, op1=ALU.max)
            # phase B: gemm2 + relu2(+bias), h2 in fp16
            for j in range(4):
                pb = ps.tile([128, 2048], F32, tag="ps")
                for s in range(4):
                    t = 4 * j + s
                    nc.tensor.matmul(
                        pb[:, 512 * s:512 * s + 512], lhsT2,
                        h1[:, 512 * t:512 * t + 512],
                        start=True, stop=True)
                dst = h2[:, 2048 * j:2048 * j + 2048]
                if j % 2 == 1:
                    nc.scalar.activation(dst, pb, ACTF.Relu, bias=b2ap, scale=1.0)
                else:
                    nc.vector.tensor_scalar(out=dst, in0=pb, scalar1=b2ap,
                                            scalar2=0.0, op0=ALU.add, op1=ALU.max)
            # phase C: gemm3 column-tiled (fp16), quadrant-packed [128, 2048]
            pc = ps.tile([128, 2048], F32, tag="ps")
            for t in range(NT):
                q, k = t % 4, t // 4
                nc.tensor.matmul(pc[32 * q:32 * q + 32, 512 * k:512 * k + 512],
                                 w3t, h2[:, 512 * t:512 * t + 512],
                                 start=True, stop=True, tile_position=(0, 32 * q))
            pred = ppool.tile([128, 2048], F32, tag="p")
            nc.scalar.activation(pred, pc, ACTF.Sigmoid, bias=b3ap, scale=1.0)
            sq = spool.tile([128, 2048], F32, tag="s")
            nc.scalar.activation(sq, pred, ACTF.Square,
                                 accum_out=psq_acc[:, u:u + 1])
            prod = spool.tile([128, 2048], F32, tag="s")
            nc.vector.scalar_tensor_tensor(
                out=prod, in0=pred, scalar=0.0, in1=tg,
                op0=ALU.add, op1=ALU.mult, accum_out=inter_acc[:, u:u + 1])

        nc.sync.dma_start(out=acc_t.ap()[0], in_=inter_acc)
        nc.sync.dma_start(out=acc_t.ap()[1], in_=psq_acc)

    nc.compile()
    _PROGRAM_CACHE[NG] = nc
    return nc


def _run(inputs, trace=False):
    seg_feat = np.asarray(inputs["seg_feat"], np.float32)
    conv_weight = np.asarray(inputs["conv_weight"], np.float32)
    mask = np.asarray(inputs["mask"])
    ind = np.asarray(inputs["ind"])
    target = np.asarray(inputs["target"], np.float32)

    in_maps, per_core, NG = host_pack(seg_feat, conv_weight, mask, ind, target)
    nc = build_program(NG)
    res = run_bass_kernel_spmd(nc, in_maps, core_ids=list(range(N_CORES)),
                               trace=trace)

    inter = np.zeros(B, np.float64)
    predsq = np.zeros(B, np.float64)
    for ci in range(N_CORES):
        acc = res.results[ci]["acc"]
        for u, (b, grp, half) in enumerate(per_core[ci]):
            if all(o < 0 for o in grp):
                continue
            inter[b] += acc[0, :, u].sum(dtype=np.float64)
            predsq[b] += acc[1, :, u].sum(dtype=np.float64)
    tgtsq = ((target.reshape(B, O, HW).astype(np.float64) ** 2)
             * mask[:, :, None]).sum(axis=(1, 2))
    loss = 1.0 - (2.0 * inter + 1.0) / (predsq + tgtsq + 1.0)
    return np.float32(loss.mean()), res


def kernel(**inputs):
    loss, _ = _run(inputs, trace=False)
    return np.array(loss, dtype=np.float32)
